# revision 3
# baseline (speedup 1.0000x reference)
"""v4: global-chunk software pipeline (g = r*NCH + c).

Same math/traffic as v3 (ids low-words, int32 out, big ops), but every engine
runs a per-chunk steady-state loop with cross-stage gating one chunk ahead, so
Pool (the longest stage) never waits for next-iteration premask:

  DVE:  pm(0) pm(1) | f1(0) f2(0) f3(0)x4 pm(2) | f1(1) f2(1) f3(1)x4 pm(3) | ...
  Pool: m2(0) m3(0) | m2(1) m3(1) | ...
  ACT:  m1(0)x16 | m1(1)x16 | ...
  sync: ld(0) ld(1) | ld(2) st(0) | ld(3) st(1) | ...
"""
import sys

for _p in ("/opt/trn_rl_repo", "/root/.axon_site/_ro/trn_rl_repo"):
    if _p not in sys.path:
        sys.path.append(_p)

import numpy as np

B, S, O, H = 64, 8192, 4, 16
NCORES = 8
BPC = B // NCORES
N = BPC * S
P = 128
KTOT = N // P                  # 512
KC = 256
NCH = KTOT // KC               # 2
NQ = 4                         # output stores per chunk
KQ = KC // NQ
TABLE = 1 << 20
MASK20 = TABLE - 1

_cache = {}


def _build(p1, p2, p3, iters=1):
    import concourse.bass as bass
    from concourse import mybir

    A = mybir.AluOpType
    I32 = mybir.dt.int32
    U8 = mybir.dt.uint8

    nc = bass.Bass()

    ids_d = nc.declare_dram_parameter("ids", [P, KTOT, 4], I32, isOutput=False)
    msk_d = nc.declare_dram_parameter("msk", [P, KTOT], U8, isOutput=False)
    cst_d = nc.declare_dram_parameter("cst", [P, 3 * H], I32, isOutput=False)
    out_d = nc.declare_dram_parameter("out", [P, KTOT, H], I32, isOutput=True)

    raw = [nc.alloc_sbuf_tensor(f"raw{c}", [P, KC, 4], I32) for c in range(NCH)]
    mk8 = [nc.alloc_sbuf_tensor(f"mk8{c}", [P, KC], U8) for c in range(NCH)]
    idm = [[nc.alloc_sbuf_tensor(f"id{i}m{c}", [P, KC], I32) for i in range(4)] for c in range(NCH)]
    cst = nc.alloc_sbuf_tensor("cst_t", [P, 3 * H], I32)
    mA = nc.alloc_sbuf_tensor("mA", [P, 1], I32)
    m1b = [nc.alloc_sbuf_tensor(f"m1b{c}", [P, H, KC], I32) for c in range(NCH)]
    m2b = [nc.alloc_sbuf_tensor(f"m2b{c}", [P, H, KC], I32) for c in range(NCH)]
    m3b = [nc.alloc_sbuf_tensor(f"m3b{c}", [P, H, KC], I32) for c in range(NCH)]
    f1b = nc.alloc_sbuf_tensor("f1b", [P, H, KC], I32)
    ot = [nc.alloc_sbuf_tensor(f"ot{c}", [P, KC, H], I32) for c in range(NCH)]

    s_in = nc.alloc_semaphore("s_in")      # +32 per chunk load pair (raw+mk)
    s_pm = nc.alloc_semaphore("s_pm")      # +1 per premask chunk done
    s_idm23 = nc.alloc_semaphore("s_idm23")
    s_idm1 = nc.alloc_semaphore("s_idm1")
    s_m1 = nc.alloc_semaphore("s_m1")
    s_m2 = nc.alloc_semaphore("s_m2")
    s_m3 = nc.alloc_semaphore("s_m3")
    s_f1 = nc.alloc_semaphore("s_f1")
    s_f2 = nc.alloc_semaphore("s_f2")
    s_f = nc.alloc_semaphore("s_f")        # +1 per q-quarter of f3
    s_out = nc.alloc_semaphore("s_out")    # +16 per store

    G = NCH * iters

    with nc.Block() as block:
        @block.sync
        def _(sync: bass.BassEngine):
            sync.dma_start(out=cst[:], in_=cst_d[:]).then_inc(s_in, 16)
            for g in range(G):
                c = g % NCH
                if g >= 2:
                    sync.wait_ge(s_pm, g - 1)
                sync.dma_start(out=raw[c][:], in_=ids_d[:, c * KC:(c + 1) * KC, :]).then_inc(s_in, 16)
                sync.dma_start(out=mk8[c][:], in_=msk_d[:, c * KC:(c + 1) * KC]).then_inc(s_in, 16)
                if g >= 1:
                    gp_, cp = g - 1, (g - 1) % NCH
                    for q in range(NQ):
                        sync.wait_ge(s_f, NQ * gp_ + q + 1)
                        sync.dma_start(
                            out=out_d[:, cp * KC + q * KQ:cp * KC + (q + 1) * KQ, :],
                            in_=ot[cp][:, q * KQ:(q + 1) * KQ, :],
                        ).then_inc(s_out, 16)
            gp_, cp = G - 1, (G - 1) % NCH
            for q in range(NQ):
                sync.wait_ge(s_f, NQ * gp_ + q + 1)
                sync.dma_start(
                    out=out_d[:, cp * KC + q * KQ:cp * KC + (q + 1) * KQ, :],
                    in_=ot[cp][:, q * KQ:(q + 1) * KQ, :],
                ).then_inc(s_out, 16)
            sync.wait_ge(s_out, 16 * NQ * G)

        def premask(v, g):
            c = g % NCH
            v.wait_ge(s_in, 16 + 32 * (g + 1))
            mseg = mk8[c][:]
            v.tensor_tensor(idm[c][2][:], raw[c][:, :, 2], mseg, A.mult).then_inc(s_idm23, 1)
            v.tensor_tensor(idm[c][3][:], raw[c][:, :, 3], mseg, A.mult).then_inc(s_idm23, 1)
            v.tensor_tensor(idm[c][1][:], raw[c][:, :, 1], mseg, A.mult).then_inc(s_idm1, 1)
            v.tensor_tensor(idm[c][0][:], raw[c][:, :, 0], mseg, A.mult).then_inc(s_pm, 1)

        @block.vector
        def _(v: bass.BassEngine):
            v.memset(mA[:], MASK20)
            premask(v, 0)
            if G > 1:
                premask(v, 1)
            for g in range(G):
                c = g % NCH
                v.wait_ge(s_m2, g + 1)
                v.wait_ge(s_m3, g + 1)
                v.scalar_tensor_tensor(f1b[:], m3b[c][:], mA[:], m2b[c][:],
                                       A.bitwise_and, A.bitwise_xor).then_inc(s_f1, 1)
                v.wait_ge(s_m1, g + 1)
                v.scalar_tensor_tensor(m3b[c][:], f1b[:], mA[:], m1b[c][:],
                                       A.bitwise_and, A.bitwise_xor).then_inc(s_f2, 1)
                for q in range(NQ):
                    if g >= 2:
                        v.wait_ge(s_out, 16 * (NQ * (g - 2) + q + 1))
                    id0q = idm[c][0][:, q * KQ:(q + 1) * KQ].rearrange(
                        "p (x k) -> p x k", x=1).broadcast_to([P, H, KQ])
                    out_ap = ot[c][:, q * KQ:(q + 1) * KQ, :].rearrange("p k h -> p h k")
                    v.scalar_tensor_tensor(out_ap, m3b[c][:, :, q * KQ:(q + 1) * KQ], mA[:],
                                           id0q, A.bitwise_and, A.bitwise_xor).then_inc(s_f, 1)
                if g + 2 < G:
                    premask(v, g + 2)

        @block.scalar
        def _(sc: bass.BassEngine):
            for g in range(G):
                c = g % NCH
                sc.wait_ge(s_idm1, g + 1)
                if g >= 2:
                    sc.wait_ge(s_f2, g - 1)
                for h in range(H):
                    ins = sc.mul(m1b[c][:, h, :], idm[c][1][:], float(p1[h]))
                    if h == H - 1:
                        ins.then_inc(s_m1, 1)

        @block.gpsimd
        def _(gp: bass.BassEngine):
            for g in range(G):
                c = g % NCH
                gp.wait_ge(s_idm23, 2 * g + 1)
                if g >= 2:
                    gp.wait_ge(s_f1, g - 1)
                i2b = idm[c][2][:].rearrange("p (x k) -> p x k", x=1).broadcast_to([P, H, KC])
                c2b = cst[:, 0:H].rearrange("p (h x) -> p h x", x=1).broadcast_to([P, H, KC])
                gp.tensor_tensor(m2b[c][:], i2b, c2b, A.mult).then_inc(s_m2, 1)
                gp.wait_ge(s_idm23, 2 * g + 2)
                if g >= 2:
                    gp.wait_ge(s_f, NQ * (g - 1))
                i3b = idm[c][3][:].rearrange("p (x k) -> p x k", x=1).broadcast_to([P, H, KC])
                c3b = cst[:, H:2 * H].rearrange("p (h x) -> p h x", x=1).broadcast_to([P, H, KC])
                gp.tensor_tensor(m3b[c][:], i3b, c3b, A.mult).then_inc(s_m3, 1)

    return nc


def _prep(ngram_ids, ngram_mask, prime_powers):
    """Shared host-side prep: per-core input maps + prime constants."""
    ids = np.asarray(ngram_ids)
    msk = np.asarray(ngram_mask)
    pw = np.asarray(prime_powers)

    p1 = [int(x) for x in pw[:H, 1]]
    p2 = [int(x) for x in pw[:H, 2]]
    p3 = [int(x & 0xFFFFFFFF) for x in pw[:H, 3]]

    ids32 = ids.view(np.int32).reshape(B, S, 2 * O)[:, :, 0::2]   # low words
    msk8 = np.ascontiguousarray(msk).astype(np.uint8, copy=False)

    cstv = np.empty((P, 3 * H), np.int32)
    cstv[:, :H] = np.asarray(p2, np.int64).astype(np.int32)[None, :]
    cstv[:, H:2 * H] = np.asarray(p3, np.uint32).view(np.int32)[None, :]
    cstv[:, 2 * H:] = np.asarray(p1, np.int64).astype(np.int32)[None, :]

    in_maps = []
    for c in range(NCORES):
        core_ids = np.ascontiguousarray(ids32[c * BPC:(c + 1) * BPC]).reshape(P, KTOT, 4)
        core_msk = np.ascontiguousarray(msk8[c * BPC:(c + 1) * BPC]).reshape(P, KTOT)
        in_maps.append({"ids": core_ids, "msk": core_msk, "cst": cstv})
    return in_maps, p1, p2, p3


def kernel(ngram_ids, ngram_mask, prime_powers, table_size):
    from concourse.bass_utils import run_bass_kernel_spmd

    assert int(table_size) == TABLE
    ids = np.asarray(ngram_ids)
    pw = np.asarray(prime_powers)
    assert ids.shape == (B, S, O) and ids.dtype == np.int64
    assert pw.shape[1] >= 4 and np.all(pw[:, 0] == 1)

    in_maps, p1, p2, p3 = _prep(ngram_ids, ngram_mask, prime_powers)

    key = (tuple(p1), tuple(p2), tuple(p3))
    if key not in _cache:
        _cache[key] = _build(p1, p2, p3)
    nc = _cache[key]

    res = run_bass_kernel_spmd(nc, in_maps, list(range(NCORES)))

    out = np.empty((B, S, H), np.int64)
    for c in range(NCORES):
        o32 = res.results[c]["out"]
        out[c * BPC:(c + 1) * BPC] = o32.reshape(BPC, S, H).astype(np.int64)
    return out


if __name__ == "__main__":
    rng = np.random.default_rng(0)
    ids = rng.integers(0, 32000, size=(B, S, O)).astype(np.int64)
    msk = np.ones((B, S), dtype=bool)
    msk[3, 100:200] = False
    primes = np.array([31, 37, 41, 43, 47, 53, 59, 61, 67, 71, 73, 79, 83, 89, 97, 101], np.int64)
    pw = primes[:, None] ** np.arange(8, dtype=np.int64)[None, :]
    got = kernel(ids, msk, pw, TABLE)
    w = ids[:, :, :, None].astype(np.int64) * pw.T[:4][None, None, :, :]
    exp = w[..., 0, :]
    for i in range(1, 4):
        exp = exp ^ w[..., i, :]
    exp = (exp % TABLE) * msk[..., None]
    print("match:", np.array_equal(got, exp))
    bad = got != exp
    if bad.any():
        idx = np.argwhere(bad)
        print("nbad:", len(idx))
        for b_, s_, h_ in idx[:5]:
            print(b_, s_, h_, got[b_, s_, h_], exp[b_, s_, h_])


# revision 4
# speedup vs baseline: 1.1538x; 1.1538x over previous
"""v5: deep-banked global-chunk pipeline (g = r*NCH + c).

v4 critical path was f3(g-2) -> st issue -> ld(g) -> premask(g) -> Pool(g):
Pool start was chained to DVE fold completion through the input load. v5
quad-buffers raw/mk/idm, issues loads 3 chunks ahead, and premasks g+2
between f1(g) and f2(g), so Pool runs back-to-back:

  DVE:  pm(0) pm(1) | f1(0) pm(2) f2(0) f3(0)x4 | f1(1) pm(3) f2(1) ... |
  Pool: m2(0) m3(0) m2(1) m3(1) ...   (continuous)
  sync: ld(0..2) | ld(3) st(0) | ld(4) st(1) | ...
"""
import sys

for _p in ("/opt/trn_rl_repo", "/root/.axon_site/_ro/trn_rl_repo"):
    if _p not in sys.path:
        sys.path.append(_p)

import numpy as np

B, S, O, H = 64, 8192, 4, 16
NCORES = 8
BPC = B // NCORES
N = BPC * S
P = 128
KTOT = N // P                  # 512
KC = 256
NCH = KTOT // KC               # 2
NQ = 4                         # output stores per chunk
KQ = KC // NQ
TABLE = 1 << 20
MASK20 = TABLE - 1

_cache = {}


def _build(p1, p2, p3, iters=1):
    import concourse.bass as bass
    from concourse import mybir

    A = mybir.AluOpType
    I32 = mybir.dt.int32
    U8 = mybir.dt.uint8

    nc = bass.Bass()

    ids_d = nc.declare_dram_parameter("ids", [P, KTOT, 4], I32, isOutput=False)
    msk_d = nc.declare_dram_parameter("msk", [P, KTOT], U8, isOutput=False)
    cst_d = nc.declare_dram_parameter("cst", [P, 3 * H], I32, isOutput=False)
    out_d = nc.declare_dram_parameter("out", [P, KTOT, H], I32, isOutput=True)

    NBUF = 4
    raw = [nc.alloc_sbuf_tensor(f"raw{c}", [P, KC, 4], I32) for c in range(NBUF)]
    mk8 = [nc.alloc_sbuf_tensor(f"mk8{c}", [P, KC], U8) for c in range(NBUF)]
    idm = [[nc.alloc_sbuf_tensor(f"id{i}m{c}", [P, KC], I32) for i in range(4)] for c in range(NBUF)]
    cst = nc.alloc_sbuf_tensor("cst_t", [P, 3 * H], I32)
    mA = nc.alloc_sbuf_tensor("mA", [P, 1], I32)
    m1b = [nc.alloc_sbuf_tensor(f"m1b{c}", [P, H, KC], I32) for c in range(NCH)]
    m2b = [nc.alloc_sbuf_tensor(f"m2b{c}", [P, H, KC], I32) for c in range(NCH)]
    m3b = [nc.alloc_sbuf_tensor(f"m3b{c}", [P, H, KC], I32) for c in range(NCH)]
    f1b = nc.alloc_sbuf_tensor("f1b", [P, H, KC], I32)
    ot = [nc.alloc_sbuf_tensor(f"ot{c}", [P, KC, H], I32) for c in range(NCH)]

    s_in = nc.alloc_semaphore("s_in")      # +32 per chunk load pair (raw+mk)
    s_pm = nc.alloc_semaphore("s_pm")      # +1 per premask chunk done
    s_idm23 = nc.alloc_semaphore("s_idm23")
    s_idm1 = nc.alloc_semaphore("s_idm1")
    s_m1 = nc.alloc_semaphore("s_m1")
    s_m2 = nc.alloc_semaphore("s_m2")
    s_m3 = nc.alloc_semaphore("s_m3")
    s_f1 = nc.alloc_semaphore("s_f1")
    s_f2 = nc.alloc_semaphore("s_f2")
    s_f = nc.alloc_semaphore("s_f")        # +1 per q-quarter of f3
    s_out = nc.alloc_semaphore("s_out")    # +16 per store

    G = NCH * iters

    with nc.Block() as block:
        @block.sync
        def _(sync: bass.BassEngine):
            sync.dma_start(out=cst[:], in_=cst_d[:]).then_inc(s_in, 16)

            def load(g):
                b, c = g % NBUF, g % NCH
                if g >= NBUF:
                    sync.wait_ge(s_pm, g - NBUF + 1)
                sync.dma_start(out=raw[b][:], in_=ids_d[:, c * KC:(c + 1) * KC, :]).then_inc(s_in, 16)
                sync.dma_start(out=mk8[b][:], in_=msk_d[:, c * KC:(c + 1) * KC]).then_inc(s_in, 16)

            def store(g):
                cp = g % NCH
                for q in range(NQ):
                    sync.wait_ge(s_f, NQ * g + q + 1)
                    sync.dma_start(
                        out=out_d[:, cp * KC + q * KQ:cp * KC + (q + 1) * KQ, :],
                        in_=ot[cp % 2][:, q * KQ:(q + 1) * KQ, :],
                    ).then_inc(s_out, 16)

            for g in range(min(3, G)):
                load(g)
            for g in range(G):
                if g + 3 < G:
                    load(g + 3)
                store(g)
            sync.wait_ge(s_out, 16 * NQ * G)

        def premask(v, g):
            b = g % NBUF
            v.wait_ge(s_in, 16 + 32 * (g + 1))
            mseg = mk8[b][:]
            v.tensor_tensor(idm[b][2][:], raw[b][:, :, 2], mseg, A.mult).then_inc(s_idm23, 1)
            v.tensor_tensor(idm[b][3][:], raw[b][:, :, 3], mseg, A.mult).then_inc(s_idm23, 1)
            v.tensor_tensor(idm[b][1][:], raw[b][:, :, 1], mseg, A.mult).then_inc(s_idm1, 1)
            v.tensor_tensor(idm[b][0][:], raw[b][:, :, 0], mseg, A.mult).then_inc(s_pm, 1)

        @block.vector
        def _(v: bass.BassEngine):
            v.memset(mA[:], MASK20)
            premask(v, 0)
            if G > 1:
                premask(v, 1)
            for g in range(G):
                c = g % NCH
                b = g % NBUF
                v.wait_ge(s_m2, g + 1)
                v.wait_ge(s_m3, g + 1)
                v.scalar_tensor_tensor(f1b[:], m3b[c][:], mA[:], m2b[c][:],
                                       A.bitwise_and, A.bitwise_xor).then_inc(s_f1, 1)
                if g + 2 < G:
                    premask(v, g + 2)
                v.wait_ge(s_m1, g + 1)
                v.scalar_tensor_tensor(m3b[c][:], f1b[:], mA[:], m1b[c][:],
                                       A.bitwise_and, A.bitwise_xor).then_inc(s_f2, 1)
                for q in range(NQ):
                    if g >= 2:
                        v.wait_ge(s_out, 16 * (NQ * (g - 2) + q + 1))
                    id0q = idm[b][0][:, q * KQ:(q + 1) * KQ].rearrange(
                        "p (x k) -> p x k", x=1).broadcast_to([P, H, KQ])
                    out_ap = ot[c][:, q * KQ:(q + 1) * KQ, :].rearrange("p k h -> p h k")
                    v.scalar_tensor_tensor(out_ap, m3b[c][:, :, q * KQ:(q + 1) * KQ], mA[:],
                                           id0q, A.bitwise_and, A.bitwise_xor).then_inc(s_f, 1)

        @block.scalar
        def _(sc: bass.BassEngine):
            for g in range(G):
                c = g % NCH
                b = g % NBUF
                sc.wait_ge(s_idm1, g + 1)
                if g >= 2:
                    sc.wait_ge(s_f2, g - 1)
                for h in range(H):
                    ins = sc.mul(m1b[c][:, h, :], idm[b][1][:], float(p1[h]))
                    if h == H - 1:
                        ins.then_inc(s_m1, 1)

        @block.gpsimd
        def _(gp: bass.BassEngine):
            for g in range(G):
                c = g % NCH
                b = g % NBUF
                gp.wait_ge(s_idm23, 2 * g + 1)
                if g >= 2:
                    gp.wait_ge(s_f1, g - 1)
                i2b = idm[b][2][:].rearrange("p (x k) -> p x k", x=1).broadcast_to([P, H, KC])
                c2b = cst[:, 0:H].rearrange("p (h x) -> p h x", x=1).broadcast_to([P, H, KC])
                gp.tensor_tensor(m2b[c][:], i2b, c2b, A.mult).then_inc(s_m2, 1)
                gp.wait_ge(s_idm23, 2 * g + 2)
                if g >= 2:
                    gp.wait_ge(s_f, NQ * (g - 1))
                i3b = idm[b][3][:].rearrange("p (x k) -> p x k", x=1).broadcast_to([P, H, KC])
                c3b = cst[:, H:2 * H].rearrange("p (h x) -> p h x", x=1).broadcast_to([P, H, KC])
                gp.tensor_tensor(m3b[c][:], i3b, c3b, A.mult).then_inc(s_m3, 1)

    return nc


def _prep(ngram_ids, ngram_mask, prime_powers):
    """Shared host-side prep: per-core input maps + prime constants."""
    ids = np.asarray(ngram_ids)
    msk = np.asarray(ngram_mask)
    pw = np.asarray(prime_powers)

    p1 = [int(x) for x in pw[:H, 1]]
    p2 = [int(x) for x in pw[:H, 2]]
    p3 = [int(x & 0xFFFFFFFF) for x in pw[:H, 3]]

    ids32 = ids.view(np.int32).reshape(B, S, 2 * O)[:, :, 0::2]   # low words
    msk8 = np.ascontiguousarray(msk).astype(np.uint8, copy=False)

    cstv = np.empty((P, 3 * H), np.int32)
    cstv[:, :H] = np.asarray(p2, np.int64).astype(np.int32)[None, :]
    cstv[:, H:2 * H] = np.asarray(p3, np.uint32).view(np.int32)[None, :]
    cstv[:, 2 * H:] = np.asarray(p1, np.int64).astype(np.int32)[None, :]

    in_maps = []
    for c in range(NCORES):
        core_ids = np.ascontiguousarray(ids32[c * BPC:(c + 1) * BPC]).reshape(P, KTOT, 4)
        core_msk = np.ascontiguousarray(msk8[c * BPC:(c + 1) * BPC]).reshape(P, KTOT)
        in_maps.append({"ids": core_ids, "msk": core_msk, "cst": cstv})
    return in_maps, p1, p2, p3


def kernel(ngram_ids, ngram_mask, prime_powers, table_size):
    from concourse.bass_utils import run_bass_kernel_spmd

    assert int(table_size) == TABLE
    ids = np.asarray(ngram_ids)
    pw = np.asarray(prime_powers)
    assert ids.shape == (B, S, O) and ids.dtype == np.int64
    assert pw.shape[1] >= 4 and np.all(pw[:, 0] == 1)

    in_maps, p1, p2, p3 = _prep(ngram_ids, ngram_mask, prime_powers)

    key = (tuple(p1), tuple(p2), tuple(p3))
    if key not in _cache:
        _cache[key] = _build(p1, p2, p3)
    nc = _cache[key]

    res = run_bass_kernel_spmd(nc, in_maps, list(range(NCORES)))

    out = np.empty((B, S, H), np.int64)
    for c in range(NCORES):
        o32 = res.results[c]["out"]
        out[c * BPC:(c + 1) * BPC] = o32.reshape(BPC, S, H).astype(np.int64)
    return out


if __name__ == "__main__":
    rng = np.random.default_rng(0)
    ids = rng.integers(0, 32000, size=(B, S, O)).astype(np.int64)
    msk = np.ones((B, S), dtype=bool)
    msk[3, 100:200] = False
    primes = np.array([31, 37, 41, 43, 47, 53, 59, 61, 67, 71, 73, 79, 83, 89, 97, 101], np.int64)
    pw = primes[:, None] ** np.arange(8, dtype=np.int64)[None, :]
    got = kernel(ids, msk, pw, TABLE)
    w = ids[:, :, :, None].astype(np.int64) * pw.T[:4][None, None, :, :]
    exp = w[..., 0, :]
    for i in range(1, 4):
        exp = exp ^ w[..., i, :]
    exp = (exp % TABLE) * msk[..., None]
    print("match:", np.array_equal(got, exp))
    bad = got != exp
    if bad.any():
        idx = np.argwhere(bad)
        print("nbad:", len(idx))
        for b_, s_, h_ in idx[:5]:
            print(b_, s_, h_, got[b_, s_, h_], exp[b_, s_, h_])
